# revision 1
# baseline (speedup 1.0000x reference)
"""Multi-head attention kernel for Trainium2, 8 NeuronCores.

Problem (hardcoded shapes): B=4, S=2048, E=1024, H=16, DH=64.
  q/k/v = einsum('bse,hed->bhsd', x, W{q,k,v}) + b{q,k,v}
  attn  = softmax(q k^T / sqrt(DH)) v
  out   = concat_heads(attn) @ Wo^T + bo

Sharding: core c -> (batch b = c//2, head-half hh = c%2, i.e. heads
8*hh..8*hh+7).  Each core computes a [S, E] partial of its batch's output
(its 512 columns of concat against the matching 512 rows of Wo^T); the host
sums the two partials per batch and adds bo.

Per-core dataflow (everything transposed so the PE contraction dim lands on
SBUF partitions):
  xT   [e=128 x 8, s=2048]  via PE-transpose of x tiles       (fp32r)
  v    [t, d'=512]          x @ Wv for all 8 heads + bias      (bf16, with a
                            fused ones column per head -> softmax sums)
  qT/kT[j=128, s=2048]      per head-pair Wq^T @ xT + bias     (fp32r);
                            next pair's projection matmuls are interleaved
                            into the current pair's attention stream so the
                            scalar engine (softmax exp, the bottleneck) never
                            starves while the PE does projections
  scoresT [t=128, s=512]    kT-block^T as lhsT, qT as rhs      (PSUM fp32)
  expST               ACT Exp(scale=1/8) on 2-bank PSUM groups (bf16)
  attnT+sums [65, s]  vext as lhsT (M=65: 64 v cols + ones)    (PSUM fp32)
  normalize           DVE recip + gpsimd partition_broadcast + DVE mul
  concatT [f=128 x 4, s]    normalized attnT                   (bf16)
  out_partial [s, e]        concatT as lhsT, Wo^T as rhs       (fp32)
"""

import os
import sys

for _p in ("/opt/trn_rl_repo", "/root/.axon_site/_ro/trn_rl_repo"):
    if os.path.isdir(_p) and _p not in sys.path:
        sys.path.insert(0, _p)
        break

from contextlib import ExitStack

import numpy as np
import ml_dtypes

import concourse.bass as bass
import concourse.tile as tile
import concourse.mybir as mybir
from concourse import bacc, bass_utils

B, S, E, H, DH = 4, 2048, 1024, 16, 64
HPC = 8           # heads per core
JW = HPC * DH     # 512, per-core qkv width
N_CORES = 8
SB = S // 128     # 16 s-blocks / t-blocks
EB = E // 128     # 8 e-blocks
SC = S // 512     # 4 s-chunks
F32 = mybir.dt.float32
F32R = mybir.dt.float32r
BF16 = mybir.dt.bfloat16
Exp = mybir.ActivationFunctionType.Exp
MULT = mybir.AluOpType.mult
ADD = mybir.AluOpType.add


def _emit(tc, aps, ctx):
    nc = tc.nc
    x_d, wq_d, wk_d, wv_d, wo_d, bqt_d, bkt_d, bv_d, id_d, out_d = aps

    def pool(**kw):
        return ctx.enter_context(tc.tile_pool(**kw))

    const = pool(name="const", bufs=1)
    xs = pool(name="xs", bufs=3)
    xTp = pool(name="xT", bufs=1)
    vxp = pool(name="vext", bufs=1)
    wqk = pool(name="wqk", bufs=2)
    qkp = pool(name="qk", bufs=2)
    exp_p = pool(name="expS", bufs=3)
    ccp = pool(name="concatT", bufs=1)
    nrm = pool(name="nrm", bufs=2)
    outp = pool(name="outs", bufs=3)
    ps_sm = pool(name="ps_sm", bufs=2, space="PSUM")
    ps_sc = pool(name="ps_sc", bufs=2, space="PSUM")
    ps_ac = pool(name="ps_ac", bufs=2, space="PSUM")

    # ---- constants / weights (x tiles are DMA'd first in emit_ab; keep the
    # bulky weight loads from queuing ahead of them) ----
    ident = const.tile([128, 128], F32R)
    nc.sync.dma_start(ident[:], id_d[:])
    bq_sb = const.tile([128, 4], F32)
    nc.sync.dma_start(bq_sb[:], bqt_d[:])
    bk_sb = const.tile([128, 4], F32)
    nc.sync.dma_start(bk_sb[:], bkt_d[:])
    bv1 = const.tile([1, JW], F32)
    nc.sync.dma_start(bv1[:], bv_d[:])
    bvb = const.tile([128, JW], F32)
    nc.gpsimd.partition_broadcast(bvb[:], bv1[:])
    wv_sb = const.tile([128, EB, JW], F32R)
    wo_sb = const.tile([128, 4, E], BF16)

    def load_wv():
        nc.sync.dma_start(wv_sb[:], wv_d.rearrange("(eb p) j -> p eb j", p=128))

    def load_wo():
        nc.sync.dma_start(wo_sb[:], wo_d.rearrange("(fb p) e -> p fb e", p=128))

    xT = xTp.tile([128, EB, S], F32R)
    vext = vxp.tile([128, SB, HPC, DH + 1], BF16)

    def dma_x(sb):
        x_t = xs.tile([128, E], F32R, tag="x_t", name=f"x_t_{sb}")
        nc.sync.dma_start(x_t[:], x_d[sb * 128:(sb + 1) * 128, :])
        return x_t

    def emit_tr(sb, x_t):
        """Transpose x s-block sb into xT."""
        for half in range(2):
            pt = ps_sm.tile([128, 512], F32R, tag="ps_sm", name=f"pt{sb}_{half}")
            for q in range(4):
                eb = half * 4 + q
                nc.tensor.transpose(pt[:, q * 128:(q + 1) * 128],
                                    x_t[:, eb * 128:(eb + 1) * 128], ident[:])
            nc.vector.tensor_copy(
                xT[:, half * 4:(half + 1) * 4, sb * 128:(sb + 1) * 128],
                pt[:].rearrange("p (e s) -> p e s", e=4))

    def emit_v(sb):
        """Project v (all 8 heads) for t-block sb into vext."""
        pv = ps_sm.tile([128, 512], F32, tag="ps_sm", name=f"pv{sb}")
        for eb in range(EB):
            nc.tensor.matmul(pv[:], xT[:, eb, sb * 128:(sb + 1) * 128],
                             wv_sb[:, eb, :],
                             start=(eb == 0), stop=(eb == EB - 1))
        nc.vector.tensor_tensor(
            vext[:, sb, :, 0:DH],
            pv[:].rearrange("p (h d) -> p h d", h=HPC),
            bvb[:].rearrange("p (h d) -> p h d", h=HPC), ADD)

    # ---- per-pair projection helpers ----
    def load_pair_weights(p):
        wq_t = wqk.tile([128, EB, 128], F32R, tag="wq")
        nc.sync.dma_start(
            wq_t[:], wq_d.rearrange("(eb pp) j -> pp eb j", pp=128)[
                :, :, p * 128:(p + 1) * 128])
        wk_t = wqk.tile([128, EB, 128], F32R, tag="wk")
        nc.sync.dma_start(
            wk_t[:], wk_d.rearrange("(eb pp) j -> pp eb j", pp=128)[
                :, :, p * 128:(p + 1) * 128])
        qT = qkp.tile([128, S], F32R, tag="qT")
        kT = qkp.tile([128, S], F32R, tag="kT")
        return wq_t, wk_t, qT, kT

    def proj_chunks(p, wq_t, wk_t, qT, kT):
        """One closure per (s-chunk, q|k): 8 matmuls + bias copy."""
        chunks = []
        for sc in range(SC):
            for w_t, dst, b_sb in ((wq_t, qT, bq_sb), (wk_t, kT, bk_sb)):
                def emit(sc=sc, w_t=w_t, dst=dst, b_sb=b_sb):
                    pq = ps_sm.tile([128, 512], F32, tag="ps_sm")
                    for eb in range(EB):
                        nc.tensor.matmul(pq[:], w_t[:, eb, :],
                                         xT[:, eb, sc * 512:(sc + 1) * 512],
                                         start=(eb == 0), stop=(eb == EB - 1))
                    nc.vector.tensor_scalar_add(
                        dst[:, sc * 512:(sc + 1) * 512], pq[:], b_sb[:, p:p + 1])
                chunks.append(emit)
        return chunks

    concatT = ccp.tile([128, 4, S], BF16)
    pair_qk = {}

    def normalize(acc, bp, p, sc):
        """attnT[d, s] / sums[s] -> concatT slice."""
        r_t = nrm.tile([1, 512], F32, tag="r")
        nc.vector.reciprocal(r_t[:], acc[64:65, :])
        rb_t = nrm.tile([64, 512], F32, tag="rb")
        nc.gpsimd.partition_broadcast(rb_t[:], r_t[:])
        st = nrm.tile([64, 512], BF16, tag="st")
        nc.vector.tensor_tensor(st[:], acc[0:64, :], rb_t[:], MULT)
        nc.sync.dma_start(
            concatT[bp:bp + 64, p, sc * 512:(sc + 1) * 512], st[:])

    def outproj_chunk(sb, ec, alt=False):
        def emit():
            pp = ps_sc if alt else ps_sm
            po = pp.tile([128, 512], F32, tag="sc" if alt else "ps_sm",
                         name=f"po_{sb}_{ec}")
            for fb in range(4):
                nc.tensor.matmul(po[:],
                                 concatT[:, fb, sb * 128:(sb + 1) * 128],
                                 wo_sb[:, fb, ec * 512:(ec + 1) * 512],
                                 start=(fb == 0), stop=(fb == 3))
            ot = outp.tile([128, 512], F32, tag="ot", name=f"ot_{sb}_{ec}")
            nc.vector.tensor_copy(ot[:], po[:])
            nc.sync.dma_start(
                out_d[sb * 128:(sb + 1) * 128, ec * 512:(ec + 1) * 512], ot[:])
        return emit

    out_chunks = []   # filled as pair-3 s-chunks complete

    # One global software-pipelined stream over every attention group:
    # attnT for group i is emitted after the scores+exp of group i+1, across
    # (pair, head, s-chunk) boundaries, so the PE never drains waiting on the
    # scalar engine at iteration boundaries.  Next pair's projections are
    # injected into the PE stream at a fixed cadence, and output-projection
    # chunks fill the PE slack during pair 3 (which has no next pair).
    INJ_EVERY = 8
    OUT_EVERY = 4

    def attn_stream():
        iters = [(p, hl, sc) for p in range(3) for hl in range(2)
                 for sc in range(SC)]
        # pair 3: s-chunk-major so outproj chunks unlock mid-pair
        iters += [(3, hl, sc) for sc in range(SC) for hl in range(2)]
        accs = {}
        pend = [None]     # (ex tile, iter key, g, acc, h)
        inj = []          # pending projection chunks for the upcoming pair
        cur_pair = -1
        gctr = 0

        def flush_pend():
            if pend[0] is None:
                return
            pex, key, pg, acc, h = pend[0]
            for t2 in range(2):
                tb = pg * 2 + t2
                nc.tensor.matmul(acc[:], vext[:, tb, h, :],
                                 pex[:, t2 * 512:(t2 + 1) * 512],
                                 start=(pg == 0 and t2 == 0),
                                 stop=(pg == 7 and t2 == 1))
            if pg == 7:
                p, hl, sc = key
                normalize(acc, hl * 64, p, sc)
                del accs[key]
                if p == 3 and hl == 1:
                    for sb in range(4 * sc, 4 * sc + 4):
                        for ec in range(2):
                            out_chunks.append(outproj_chunk(sb, ec))
            pend[0] = None

        for key in iters:
            p, hl, sc = key
            if p != cur_pair:
                cur_pair = p
                if p == 2:
                    load_wo()
                if p + 1 < 4:
                    nstate = load_pair_weights(p + 1)
                    inj.extend(proj_chunks(p + 1, *nstate))
                    pair_qk[p + 1] = (nstate[2], nstate[3])
            qT, kT = pair_qk[p]
            h = p * 2 + hl
            bp = hl * 64
            qs = qT[bp:bp + 64, sc * 512:(sc + 1) * 512]
            accs[key] = ps_ac.tile([65, 512], F32, tag="acc",
                                   name=f"acc_{p}_{hl}_{sc}")
            for g in range(8):
                scp = ps_sc.tile([128, 1024], F32, tag="sc",
                                 name=f"scp_{p}_{hl}_{sc}_{g}")
                for t2 in range(2):
                    tb = g * 2 + t2
                    nc.tensor.matmul(scp[:, t2 * 512:(t2 + 1) * 512],
                                     kT[bp:bp + 64, tb * 128:(tb + 1) * 128],
                                     qs, start=True, stop=True)
                ex = exp_p.tile([128, 1024], BF16, tag="ex",
                                name=f"ex_{p}_{hl}_{sc}_{g}")
                nc.scalar.activation(ex[:], scp[:], Exp, scale=0.125)
                flush_pend()
                pend[0] = (ex, key, g, accs[key], h)
                gctr += 1
                if inj and gctr % INJ_EVERY == 0:
                    inj.pop(0)()
                elif out_chunks and gctr % OUT_EVERY == 0:
                    out_chunks.pop(0)()
                yield
        flush_pend()
        for ch in inj:  # leftovers, if any
            ch()

    # ---- prefix: transposes + v, with pair-0 projections and the first
    # attention groups interleaved so the scalar engine starts early ----
    nc.gpsimd.memset(vext[:, :, :, DH:DH + 1], 1.0)
    x0 = dma_x(0)
    load_wv()
    state = load_pair_weights(0)
    p0_chunks = proj_chunks(0, *state)  # [sc0q, sc0k, sc1q, sc1k, ...]
    pair_qk[0] = (state[2], state[3])
    gen = attn_stream()

    def pump(n):
        for _ in range(n):
            next(gen, None)

    x_tiles = {0: x0, 1: dma_x(1)}
    for sb in range(SB):
        if sb + 2 < SB:
            x_tiles[sb + 2] = dma_x(sb + 2)  # 2-deep DMA prefetch
        emit_tr(sb, x_tiles.pop(sb))
        if sb >= 2:
            emit_v(sb - 2)           # lag v so the wv DMA has landed
        if sb % 4 == 3:
            ch = sb // 4
            p0_chunks[2 * ch]()      # q chunk sc=ch
            p0_chunks[2 * ch + 1]()  # k chunk sc=ch
        if sb >= 4 and sb % 2 == 0:
            pump(1)                  # groups 0..5 at sb 4,6,8,10,12,14
    emit_v(SB - 2)
    emit_v(SB - 1)
    for _ in gen:
        pass

    # ---- remaining output-projection chunks (alternate PSUM pools so four
    # chunks can be in flight; the scores pool is idle by now) ----
    pairs = [(sb, ec) for sc in range(SC) for sb in range(4 * sc, 4 * sc + 4)
             for ec in range(2)]
    rem = pairs[-len(out_chunks):] if out_chunks else []
    for i, (sb, ec) in enumerate(rem):
        outproj_chunk(sb, ec, alt=(i % 2 == 1))()


_CACHE = {}


def _build():
    nc = bacc.Bacc("TRN2", target_bir_lowering=False, debug=False,
                   num_devices=N_CORES)
    x_d = nc.dram_tensor("x", [S, E], F32R, kind="ExternalInput").ap()
    wq_d = nc.dram_tensor("wq", [E, JW], F32R, kind="ExternalInput").ap()
    wk_d = nc.dram_tensor("wk", [E, JW], F32R, kind="ExternalInput").ap()
    wv_d = nc.dram_tensor("wv", [E, JW], F32R, kind="ExternalInput").ap()
    wo_d = nc.dram_tensor("wo", [JW, E], BF16, kind="ExternalInput").ap()
    bqt_d = nc.dram_tensor("bqt", [128, 4], F32, kind="ExternalInput").ap()
    bkt_d = nc.dram_tensor("bkt", [128, 4], F32, kind="ExternalInput").ap()
    bv_d = nc.dram_tensor("bv", [1, JW], F32, kind="ExternalInput").ap()
    id_d = nc.dram_tensor("ident", [128, 128], F32R, kind="ExternalInput").ap()
    out_d = nc.dram_tensor("out", [S, E], F32, kind="ExternalOutput").ap()
    aps = (x_d, wq_d, wk_d, wv_d, wo_d, bqt_d, bkt_d, bv_d, id_d, out_d)
    with tile.TileContext(nc) as tc:
        with ExitStack() as ctx:
            _emit(tc, aps, ctx)
    nc.compile()
    return nc


def kernel(x, Wq, bq, Wk, bk, Wv, bv, Wo, bo):
    x = np.asarray(x, dtype=np.float32)
    Wq = np.asarray(Wq, dtype=np.float32)
    bq = np.asarray(bq, dtype=np.float32)
    Wk = np.asarray(Wk, dtype=np.float32)
    bk = np.asarray(bk, dtype=np.float32)
    Wv = np.asarray(Wv, dtype=np.float32)
    bv = np.asarray(bv, dtype=np.float32)
    Wo = np.asarray(Wo, dtype=np.float32)
    bo = np.asarray(bo, dtype=np.float32)

    if "nc" not in _CACHE:
        _CACHE["nc"] = _build()
    nc = _CACHE["nc"]

    WoT = np.ascontiguousarray(Wo.T)  # [f, e]
    in_maps = []
    for c in range(N_CORES):
        b, hh = c // 2, c % 2
        hs = slice(hh * HPC, (hh + 1) * HPC)
        in_maps.append({
            "x": np.ascontiguousarray(x[b]),
            "wq": np.ascontiguousarray(
                Wq[hs].transpose(1, 0, 2).reshape(E, JW)),
            "wk": np.ascontiguousarray(
                Wk[hs].transpose(1, 0, 2).reshape(E, JW)),
            "wv": np.ascontiguousarray(
                Wv[hs].transpose(1, 0, 2).reshape(E, JW)),
            "wo": np.ascontiguousarray(
                WoT[hh * JW:(hh + 1) * JW]).astype(ml_dtypes.bfloat16),
            "bqt": np.ascontiguousarray(bq[hs].reshape(4, 128).T),
            "bkt": np.ascontiguousarray(bk[hs].reshape(4, 128).T),
            "bv": bv[hs].reshape(1, JW),
            "ident": np.eye(128, dtype=np.float32),
        })

    res = bass_utils.run_bass_kernel_spmd(nc, in_maps,
                                          core_ids=list(range(N_CORES)))
    out = np.empty((B, S, E), dtype=np.float32)
    for b in range(B):
        out[b] = res.results[2 * b]["out"] + res.results[2 * b + 1]["out"]
        out[b] += bo[None, :]
    return out



# revision 21
# speedup vs baseline: 1.2322x; 1.2322x over previous
"""Multi-head attention kernel for Trainium2, 8 NeuronCores.

Problem (hardcoded shapes): B=4, S=2048, E=1024, H=16, DH=64.
  q/k/v = einsum('bse,hed->bhsd', x, W{q,k,v}) + b{q,k,v}
  attn  = softmax(q k^T / sqrt(DH)) v
  out   = concat_heads(attn) @ Wo^T + bo

Sharding: core c -> (batch b = c//2, head-half hh = c%2, i.e. heads
8*hh..8*hh+7).  Each core computes a [S, E] partial of its batch's output;
the host sums the two partials per batch, adds bo and the (linear) v-bias
contribution bv @ Wo^T.

Design: fp8e4m3 DoubleRow matmuls (0.5 PE cycles/row, 2 k-tiles per
instruction) with residual compensation to stay inside the 2e-2 gate:
  - x is shipped as xT8 + xTr8 (two-term fp8 = ~bf16 accuracy); W as
    W8 + Wr8.  Projections run 3 DoubleRow passes (x8@W8, xr8@W8, x8@Wr8)
    plus a rank-1 ones-row matmul that adds the q/k bias exactly.
  - q is stored as (q8, qr8) in the two DoubleRow k-tile slots of the
    scores matmul (k8 duplicated via SBUF->SBUF DMA), so scores see
    full-precision q against fp8 k at no extra PE cost.
  - v is stored as v8 + vr8; the attention matmul runs two DoubleRow
    instructions per t-pair accumulating into the same PSUM rows.
  - Softmax exp (33.5M elts/core, the engine-time floor) is split across
    ACT (true Exp) and DVE/GPSIMD (Schraudolph int8 exp: y = x*a+b
    converted to int8, bitcast to fp8 -- the integer IS the fp8 bits),
    load-balanced greedily with cost-model rates.
  - attn^T accumulates [65, 512] (ones column -> softmax sums in row 64);
    normalize = DVE reciprocal + gpsimd partition_broadcast + DVE multiply
    into bf16 concatT; output projection is plain bf16 matmuls.
"""

import os
import sys
import math

for _p in ("/opt/trn_rl_repo", "/root/.axon_site/_ro/trn_rl_repo"):
    if os.path.isdir(_p) and _p not in sys.path:
        sys.path.insert(0, _p)
        break

from collections import deque
from contextlib import ExitStack

import numpy as np
import ml_dtypes

import concourse.bass as bass
import concourse.tile as tile
import concourse.mybir as mybir
from concourse import bacc, bass_utils

B, S, E, H, DH = 4, 2048, 1024, 16, 64
HPC = 8           # heads per core
JW = HPC * DH     # 512, per-core qkv width
N_CORES = 8
SB = S // 128     # 16 t-blocks
SC = S // 512     # 4 s-chunks
KB = 8            # e-blocks (contraction)
F32 = mybir.dt.float32
BF16 = mybir.dt.bfloat16
FP8 = mybir.dt.float8e4
I8 = mybir.dt.int8
Exp = mybir.ActivationFunctionType.Exp
Copy = mybir.ActivationFunctionType.Copy
MULT = mybir.AluOpType.mult
ADD = mybir.AluOpType.add
SUB = mybir.AluOpType.subtract
DR = mybir.MatmulPerfMode.DoubleRow

# scales: weights carried x32 in fp8; q/k written /16 (= 2x natural) so
# scores psum = 4*q.k; exp scale folds the /4 and the 1/sqrt(DH).
W_SCALE = 32.0
QK_OUT = 1.0 / 16.0
V_OUT = 1.0 / 32.0
EXP_SCALE = 0.125 / 4.0
# Schraudolph: int8 bits of fp8e4m3(exp(x*EXP_SCALE)) ~= round(x*A + B)
SCH_A = 8.0 * EXP_SCALE / math.log(2.0)
SCH_B = 56.0 - 0.35

LAG = 2           # attn matmuls lag their exp by LAG g-units
EX_BUFS = 4       # ex tile ring depth (attn operand buffering)
PS_BUFS = 6       # PSUM [128,512] pool (scores/proj/outproj), 1 bank each


class EngineSched:
    """Greedy static balancer over ACT/DVE/Pool using cost-model rates."""

    def __init__(self):
        self.t = {"A": 0.0, "D": 0.0, "P": 0.0}

    def _cost(self, e, cols):
        if e == "A":
            return cols * 0.8333 + 143.0
        if e == "D":
            return cols * 1.0417 + 125.0
        return cols * 1.3889 + 95.0

    def pick(self, allowed, cols):
        e = min(allowed, key=lambda x: self.t[x] + self._cost(x, cols))
        self.t[e] += self._cost(e, cols)
        return e

    def charge(self, e, cols):
        self.t[e] += self._cost(e, cols)


def _emit(tc, aps, ctx):
    nc = tc.nc
    (xt_d, xtr_d, wq_d, wqr_d, wk_d, wkr_d, wv_d, wvr_d, wo_d,
     bq_d, bk_d, out_d) = aps
    sched = EngineSched()

    def pool(**kw):
        return ctx.enter_context(tc.tile_pool(**kw))

    const = pool(name="const", bufs=1)
    xTp = pool(name="xT", bufs=1)
    qkp = pool(name="qk", bufs=1)
    vxp = pool(name="vext", bufs=1)
    ccp = pool(name="concatT", bufs=1)
    exp_p = pool(name="ex", bufs=EX_BUFS)
    nrm = pool(name="nrm", bufs=2)
    outp = pool(name="outs", bufs=3)
    ps = pool(name="ps", bufs=PS_BUFS, space="PSUM")
    psa = pool(name="psacc", bufs=2, space="PSUM")

    # ---- constants / weights ----
    bq_sb = const.tile([1, JW], FP8)
    nc.sync.dma_start(bq_sb[:], bq_d[:])
    bk_sb = const.tile([1, JW], FP8)
    nc.sync.dma_start(bk_sb[:], bk_d[:])
    ones_r = const.tile([1, 512], FP8)
    nc.vector.memset(ones_r[:], 1.0)

    w_sb = {}
    for nm, d in (("q", wq_d), ("qr", wqr_d), ("k", wk_d), ("kr", wkr_d),
                  ("v", wv_d), ("vr", wvr_d)):
        w_sb[nm] = const.tile([128, KB, JW], FP8, name=f"w{nm}")
    wo_sb = const.tile([128, 4, E], BF16)

    xT = xTp.tile([128, KB, S], FP8)
    xTr = xTp.tile([128, KB, S], FP8, name="xTr")
    # q/k per head-pair: [part = (h%2)*64 + d, slot, s]; q slots = (q8, qr8),
    # k slots = (k8, k8 duplicate) so scores DR contracts (q8+qr8) vs k8.
    qT = [qkp.tile([128, 2, S], FP8, name=f"qT{p}") for p in range(4)]
    kT = [qkp.tile([128, 2, S], FP8, name=f"kT{p}") for p in range(4)]
    # per-head v row padded to 80 so the DoubleRow weights AP k-tile
    # step (HPC*VP) is a multiple of 16 (s3_lw dual-fp8 ISA rule)
    VP = 80
    vext = vxp.tile([128, SB, HPC, VP], FP8)
    vextR = vxp.tile([128, SB, HPC, VP], FP8, name="vextR")
    concatT = ccp.tile([128, 4, S], BF16)

    # spread input loads over four DMA queues so the first projection can
    # start ~3.5us in instead of ~17us (single-queue serialization)
    for si in range(SC):
        for t_, d_ in ((xT, xt_d), (xTr, xtr_d)):
            nc.sync.dma_start(
                t_[:, :, si * 512:(si + 1) * 512],
                d_.rearrange("(eb p) s -> p eb s", p=128)[
                    :, :, si * 512:(si + 1) * 512])
    qmap = {"k": nc.scalar, "kr": nc.scalar, "q": nc.gpsimd,
            "qr": nc.gpsimd, "v": nc.gpsimd, "vr": nc.gpsimd}
    for nm, d in (("k", wk_d), ("q", wq_d), ("v", wv_d), ("kr", wkr_d),
                  ("qr", wqr_d), ("vr", wvr_d)):
        qmap[nm].dma_start(w_sb[nm][:],
                           d.rearrange("(eb p) j -> p eb j", p=128))
    nc.scalar.dma_start(wo_sb[:],
                        wo_d.rearrange("(fb p) e -> p fb e", p=128))
    nc.vector.memset(vext[:, :, :, DH:DH + 1], 1.0)

    # ---- engine-dispatched element ops ----
    # NOTE: GPSIMD/Pool cannot access PSUM (BIR verifier), so every op that
    # reads a matmul result is ACT or DVE; Pool gets the SBUF-side work.
    def d_exp(dst, scp):
        e = sched.pick("AD", 512)
        if e == "A":
            nc.scalar.activation(dst, scp[:], Exp, scale=EXP_SCALE)
        else:
            nc.vector.tensor_scalar(dst.bitcast(I8), scp[:], SCH_A, SCH_B,
                                    MULT, ADD)

    def d_ts(dst, src, scale):
        e = sched.pick("AD", 512)
        if e == "A":
            nc.scalar.activation(dst, src, Copy, scale=scale)
        else:
            nc.vector.tensor_scalar(dst, src, scale, None, MULT)

    def d_res(dst, src, scale, base):
        """dst = fp8(src*scale - base); needs a tensor operand -> DVE only"""
        sched.charge("D", 512)
        nc.vector.scalar_tensor_tensor(dst, src, scale, base, MULT, SUB)

    def d_outcopy(dst, po):
        e = sched.pick("AD", 512)
        if e == "A":
            nc.scalar.activation(dst, po[:], Copy)
        else:
            nc.vector.tensor_copy(dst, po[:])

    # ---- PE work units ----
    def proj_qk(wn, wrn, dT, b_sb, pair, sc, dup_k):
        def emit():
            pq = ps.tile([128, 512], F32, tag="ps", name=f"p{wn}{pair}_{sc}")
            wslice = slice(pair * 128, (pair + 1) * 128)
            ss = slice(sc * 512, (sc + 1) * 512)
            plan = [(xT, w_sb[wn]), (xTr, w_sb[wn]), (xT, w_sb[wrn])]
            n = 0
            for rhs_t, w in plan:
                for kb in range(4):
                    n += 1
                    nc.tensor.matmul(pq[:],
                                     w[:, 2 * kb:2 * kb + 2, wslice],
                                     rhs_t[:, 2 * kb:2 * kb + 2, ss],
                                     start=(n == 1), stop=False,
                                     perf_mode=DR)
            # rank-1 bias row (exact bias, fp8 noise ~0.1%): += 32b ⊗ ones
            nc.tensor.matmul(pq[:], b_sb[:, wslice], ones_r[:],
                             start=False, stop=True)
            dst0 = dT[pair][:, 0, sc * 512:(sc + 1) * 512]
            d_ts(dst0, pq[:], QK_OUT)
            if dup_k:
                nc.sync.dma_start(
                    dT[pair][:, 1, sc * 512:(sc + 1) * 512], dst0)
            else:
                d_res(dT[pair][:, 1, sc * 512:(sc + 1) * 512], pq[:],
                      QK_OUT, dst0)
        return emit

    def v_chunk(tb):
        def emit():
            pv = ps.tile([128, 512], F32, tag="ps", name=f"pv{tb}")
            ts_ = slice(tb * 128, (tb + 1) * 128)
            plan = [(xT, w_sb["v"]), (xTr, w_sb["v"]), (xT, w_sb["vr"])]
            n = 0
            for lhs_t, w in plan:
                for kb in range(4):
                    n += 1
                    nc.tensor.matmul(pv[:],
                                     lhs_t[:, 2 * kb:2 * kb + 2, ts_],
                                     w[:, 2 * kb:2 * kb + 2, :],
                                     start=(n == 1), stop=(n == 12),
                                     perf_mode=DR)
            dst0 = vext[:, tb, :, 0:DH]
            pvr = pv[:].rearrange("p (h d) -> p h d", h=HPC)
            d_ts(dst0, pvr, V_OUT)
            d_res(vextR[:, tb, :, 0:DH], pvr, V_OUT, dst0)
        return emit

    def outproj_chunk(sb, ec):
        def emit():
            po = ps.tile([128, 512], F32, tag="ps", name=f"po{sb}_{ec}")
            for fb in range(4):
                nc.tensor.matmul(po[:],
                                 concatT[:, fb, sb * 128:(sb + 1) * 128],
                                 wo_sb[:, fb, ec * 512:(ec + 1) * 512],
                                 start=(fb == 0), stop=(fb == 3))
            ot = outp.tile([128, 512], F32, tag="ot", name=f"ot{sb}_{ec}")
            d_outcopy(ot[:], po)
            nc.sync.dma_start(
                out_d[sb * 128:(sb + 1) * 128, ec * 512:(ec + 1) * 512],
                ot[:])
        return emit

    def normalize(acc, h, sc):
        # hop 1 frees the PSUM acc bank fast; the rest is slack-tolerant
        # (concatT is only read by outproj at s-chunk end).
        accS = nrm.tile([DH + 1, 512], F32, tag="accS")
        d_outcopy(accS[:], acc)
        r_t = nrm.tile([1, 512], F32, tag="r")
        nc.vector.reciprocal(r_t[:], accS[DH:DH + 1, :])
        rb_t = nrm.tile([64, 512], F32, tag="rb")
        nc.gpsimd.partition_broadcast(rb_t[:], r_t[:])
        sched.charge("D", 512)
        sched.charge("P", 512)
        bp = (h % 2) * 64
        nc.gpsimd.tensor_tensor(
            concatT[bp:bp + 64, h // 2, sc * 512:(sc + 1) * 512],
            accS[0:DH, :], rb_t[:], MULT)
        sched.charge("P", 512)
        sched.charge("P", 512)  # Multiply runs at 0.42 eff on Q7

    # ---- chunk registry: emit-on-demand + background fill order ----
    chunks = {}
    fill_order = deque()

    def reg(key, fn):
        chunks[key] = fn
        fill_order.append(key)

    def ensure(key):
        fn = chunks.pop(key, None)
        if fn is not None:
            fn()

    def fill(n):
        for _ in range(n):
            while fill_order and fill_order[0] not in chunks:
                fill_order.popleft()
            if not fill_order:
                return
            ensure(fill_order.popleft())

    for scp_ in range(SC):
        for pair in range(4):
            reg(("k", pair, scp_),
                proj_qk("k", "kr", kT, bk_sb, pair, scp_, True))
            reg(("q", pair, scp_),
                proj_qk("q", "qr", qT, bq_sb, pair, scp_, False))
    for tb in range(SB):
        reg(("v", tb), v_chunk(tb))

    def ensure_qk(pair, sc_q, tbmax):
        ensure(("q", pair, sc_q))
        for scp_ in range(tbmax // 4 + 1):
            ensure(("k", pair, scp_))

    # ---- main attention stream: s-chunk outer, head inner ----
    pend = deque()

    def attn_step(acc, ex_t, h, g, sc):
        def emit():
            ensure(("v", 2 * g))
            ensure(("v", 2 * g + 1))
            # main (v8+ones) instr opens the group at g0 (zeroing the full
            # bank incl. the sums row) and closes it at g7; the residual
            # (vr8) instr accumulates rows 0..63 strictly inside the group.
            a_args = dict(start=(g == 0), stop=(g == 7), perf_mode=DR,
                          skip_group_check=True)
            b_args = dict(start=False, stop=False, perf_mode=DR,
                          skip_group_check=True)
            if g == 7:
                nc.tensor.matmul(acc[0:DH, :],
                                 vextR[:, 2 * g:2 * g + 2, h, 0:DH],
                                 ex_t[:], **b_args)
                nc.tensor.matmul(acc[:], vext[:, 2 * g:2 * g + 2, h, 0:DH + 1],
                                 ex_t[:], **a_args)
            else:
                nc.tensor.matmul(acc[:], vext[:, 2 * g:2 * g + 2, h, 0:DH + 1],
                                 ex_t[:], **a_args)
                nc.tensor.matmul(acc[0:DH, :],
                                 vextR[:, 2 * g:2 * g + 2, h, 0:DH],
                                 ex_t[:], **b_args)
            if g == 7:
                normalize(acc, h, sc)
                if h == HPC - 1:
                    for sb in range(4 * sc, 4 * sc + 4):
                        reg(("o", sb, 0), outproj_chunk(sb, 0))
                        reg(("o", sb, 1), outproj_chunk(sb, 1))
        return emit

    # head-outer spreads the k/v projection front-load across the run
    # (pair p's k chunks are pulled by head 2p); out-projections still
    # unlock per s-chunk because head 7 runs last.
    for h in range(HPC):
        for sc in range(SC):
            pair, bp = h // 2, (h % 2) * 64
            acc = psa.tile([DH + 1, 512], F32, tag="acc",
                           name=f"acc{h}_{sc}")
            for g in range(8):
                ex_t = exp_p.tile([128, 2, 512], FP8, tag="ex",
                                  name=f"ex{h}_{sc}_{g}")
                for t2 in range(2):
                    tb = 2 * g + t2
                    ensure_qk(pair, sc, tb)
                    scp = ps.tile([128, 512], F32, tag="ps",
                                  name=f"scp{h}_{sc}_{tb}")
                    nc.tensor.matmul(
                        scp[:],
                        kT[pair][bp:bp + 64, :, tb * 128:(tb + 1) * 128],
                        qT[pair][bp:bp + 64, :, sc * 512:(sc + 1) * 512],
                        start=True, stop=True, perf_mode=DR)
                    d_exp(ex_t[:, t2, :], scp)
                pend.append(attn_step(acc, ex_t, h, g, sc))
                fill(1)
                while len(pend) > LAG:
                    pend.popleft()()
    while pend:
        pend.popleft()()
    fill(len(fill_order))


_CACHE = {}


def _build():
    nc = bacc.Bacc("TRN2", target_bir_lowering=False, debug=False,
                   num_devices=N_CORES)
    names = [("xt", [E, S], FP8), ("xtr", [E, S], FP8),
             ("wq", [E, JW], FP8), ("wqr", [E, JW], FP8),
             ("wk", [E, JW], FP8), ("wkr", [E, JW], FP8),
             ("wv", [E, JW], FP8), ("wvr", [E, JW], FP8),
             ("wo", [JW, E], BF16),
             ("bq", [1, JW], FP8), ("bk", [1, JW], FP8)]
    aps = tuple(nc.dram_tensor(n, s, d, kind="ExternalInput").ap()
                for n, s, d in names)
    out_d = nc.dram_tensor("out", [S, E], F32, kind="ExternalOutput").ap()
    with tile.TileContext(nc) as tc:
        with ExitStack() as ctx:
            _emit(tc, aps + (out_d,), ctx)
    nc.compile()
    return nc


FP8NP = ml_dtypes.float8_e4m3
BF16NP = ml_dtypes.bfloat16


def _two_term(a):
    """fp8 value + fp8 residual of array a."""
    a8 = a.astype(FP8NP)
    r8 = (a - a8.astype(np.float32)).astype(FP8NP)
    return a8, r8


def kernel(x, Wq, bq, Wk, bk, Wv, bv, Wo, bo):
    x = np.asarray(x, dtype=np.float32)
    Wq = np.asarray(Wq, dtype=np.float32)
    bq = np.asarray(bq, dtype=np.float32)
    Wk = np.asarray(Wk, dtype=np.float32)
    bk = np.asarray(bk, dtype=np.float32)
    Wv = np.asarray(Wv, dtype=np.float32)
    bv = np.asarray(bv, dtype=np.float32)
    Wo = np.asarray(Wo, dtype=np.float32)
    bo = np.asarray(bo, dtype=np.float32)

    if "nc" not in _CACHE:
        _CACHE["nc"] = _build()
    nc = _CACHE["nc"]

    WoT = np.ascontiguousarray(Wo.T)  # [f, e]
    # v-bias is linear through attention: its output contribution is the
    # constant row bv_flat @ Wo^T, added host-side.
    badj = bo + bv.reshape(H * DH) @ WoT

    in_maps = []
    for c in range(N_CORES):
        b, hh = c // 2, c % 2
        hs = slice(hh * HPC, (hh + 1) * HPC)
        xt8, xtr8 = _two_term(np.ascontiguousarray(x[b].T))
        m = {"xt": xt8, "xtr": xtr8,
             "wo": np.ascontiguousarray(
                 WoT[hh * JW:(hh + 1) * JW]).astype(BF16NP),
             "bq": (W_SCALE * bq[hs].reshape(1, JW)).astype(FP8NP),
             "bk": (W_SCALE * bk[hs].reshape(1, JW)).astype(FP8NP)}
        for nm, W in (("q", Wq), ("k", Wk), ("v", Wv)):
            flat = np.ascontiguousarray(
                W[hs].transpose(1, 0, 2).reshape(E, JW)) * W_SCALE
            m["w" + nm], m["w" + nm + "r"] = _two_term(flat)
        in_maps.append(m)

    res = bass_utils.run_bass_kernel_spmd(nc, in_maps,
                                          core_ids=list(range(N_CORES)))
    out = np.empty((B, S, E), dtype=np.float32)
    for b in range(B):
        out[b] = res.results[2 * b]["out"] + res.results[2 * b + 1]["out"]
        out[b] += badj[None, :]
    return out


# revision 35
# speedup vs baseline: 1.2931x; 1.0494x over previous
"""Multi-head attention kernel for Trainium2, 8 NeuronCores.

Problem (hardcoded shapes): B=4, S=2048, E=1024, H=16, DH=64.
  q/k/v = einsum('bse,hed->bhsd', x, W{q,k,v}) + b{q,k,v}
  attn  = softmax(q k^T / sqrt(DH)) v
  out   = concat_heads(attn) @ Wo^T + bo

Sharding: core c -> (batch b = c//2, head-half hh = c%2, i.e. heads
8*hh..8*hh+7).  Each core computes a [S, E] partial of its batch's output;
the host sums the two partials per batch, adds bo and the (linear) v-bias
contribution bv @ Wo^T.

Design: fp8e4m3 DoubleRow matmuls (0.5 PE cycles/row, 2 k-tiles per
instruction) with residual compensation to stay inside the 2e-2 gate:
  - x is shipped as xT8 + xTr8 (two-term fp8 = ~bf16 accuracy); W as
    W8 + Wr8.  Projections run 3 DoubleRow passes (x8@W8, xr8@W8, x8@Wr8)
    plus a rank-1 ones-row matmul that adds the q/k bias exactly.
  - q is stored as (q8, qr8) in the two DoubleRow k-tile slots of the
    scores matmul (k8 duplicated via SBUF->SBUF DMA), so scores see
    full-precision q against fp8 k at no extra PE cost.
  - v is stored as v8 + vr8; the attention matmul runs two DoubleRow
    instructions per t-pair accumulating into the same PSUM rows.
  - Softmax exp (33.5M elts/core, the engine-time floor) is split across
    ACT (true Exp) and DVE/GPSIMD (Schraudolph int8 exp: y = x*a+b
    converted to int8, bitcast to fp8 -- the integer IS the fp8 bits),
    load-balanced greedily with cost-model rates.
  - attn^T accumulates [65, 512] (ones column -> softmax sums in row 64);
    normalize = DVE reciprocal + gpsimd partition_broadcast + DVE multiply
    into bf16 concatT; output projection is plain bf16 matmuls.
"""

import os
import sys
import math

for _p in ("/opt/trn_rl_repo", "/root/.axon_site/_ro/trn_rl_repo"):
    if os.path.isdir(_p) and _p not in sys.path:
        sys.path.insert(0, _p)
        break

from collections import deque
from contextlib import ExitStack

import numpy as np
import ml_dtypes

import concourse.bass as bass
import concourse.tile as tile
import concourse.mybir as mybir
from concourse import bacc, bass_utils

B, S, E, H, DH = 4, 2048, 1024, 16, 64
HPC = 8           # heads per core
JW = HPC * DH     # 512, per-core qkv width
N_CORES = 8
SB = S // 128     # 16 t-blocks
SC = S // 512     # 4 s-chunks
KB = 8            # e-blocks (contraction)
F32 = mybir.dt.float32
BF16 = mybir.dt.bfloat16
FP8 = mybir.dt.float8e4
I8 = mybir.dt.int8
Exp = mybir.ActivationFunctionType.Exp
Copy = mybir.ActivationFunctionType.Copy
MULT = mybir.AluOpType.mult
ADD = mybir.AluOpType.add
SUB = mybir.AluOpType.subtract
DR = mybir.MatmulPerfMode.DoubleRow

# scales: weights carried x32 in fp8; q/k written /16 (= 2x natural) so
# scores psum = 4*q.k; exp scale folds the /4 and the 1/sqrt(DH).
W_SCALE = 32.0
QK_OUT = 1.0 / 16.0
V_OUT = 1.0 / 32.0
EXP_SCALE = 0.125 / 4.0
# Schraudolph: int8 bits of fp8e4m3(exp(x*EXP_SCALE)) ~= round(x*A + B)
SCH_A = 8.0 * EXP_SCALE / math.log(2.0)
SCH_B = 56.0 - 0.35

LAG = 2           # attn matmuls lag their exp by LAG g-units
EXP_ENGINES = "AD"  # which engines run exp (probe knob)
WARM_UNITS = 20   # first g-units force exp onto ACT (keep DVE for copies)
EX_BUFS = 4       # ex tile ring depth (attn operand buffering)
PS_BUFS = 6       # PSUM [128,512] pool (scores/proj/outproj), 1 bank each
SCP_PAIR = False  # scores into [128,1024] 2-bank tiles; one exp instr per pair
ATTN_SMAJ = True  # s-major attention (acc [s,65]x4/bank, Pool normalize_recip)
SCP2_BUFS = 2     # bufs for the paired scp tag (2 banks each)


class EngineSched:
    """Greedy static balancer over ACT/DVE/Pool using cost-model rates."""

    def __init__(self):
        self.t = {"A": 0.0, "D": 0.0, "P": 0.0}

    def _cost(self, e, cols):
        if e == "A":
            return cols * 0.8333 + 185.0
        if e == "D":
            return cols * 1.0417 + 125.0
        return cols * 1.3889 + 95.0

    def pick(self, allowed, cols):
        e = min(allowed, key=lambda x: self.t[x] + self._cost(x, cols))
        self.t[e] += self._cost(e, cols)
        return e

    def charge(self, e, cols):
        self.t[e] += self._cost(e, cols)


def _emit(tc, aps, ctx):
    nc = tc.nc
    (xt_d, xtr_d, wq_d, wqr_d, wk_d, wkr_d, wv_d, wvr_d, wo_d,
     bq_d, bk_d, id_d, out_d) = aps
    sched = EngineSched()

    def pool(**kw):
        return ctx.enter_context(tc.tile_pool(**kw))

    const = pool(name="const", bufs=1)
    xTp = pool(name="xT", bufs=1)
    qkp = pool(name="qk", bufs=1)
    vxp = pool(name="vext", bufs=1)
    ccp = pool(name="concatT", bufs=1)
    exp_p = pool(name="ex", bufs=EX_BUFS)
    nrm = pool(name="nrm", bufs=2)
    outp = pool(name="outs", bufs=3)
    ps = pool(name="ps", bufs=PS_BUFS, space="PSUM")
    psa = pool(name="psacc", bufs=2, space="PSUM")

    # ---- constants / weights ----
    bq_sb = const.tile([1, JW], FP8)
    bk_sb = const.tile([1, JW], FP8)
    ones_r = const.tile([1, 512], FP8)
    nc.vector.memset(ones_r[:], 1.0)
    ident = const.tile([128, 128], BF16)

    w_sb = {}
    for nm, d in (("q", wq_d), ("qr", wqr_d), ("k", wk_d), ("kr", wkr_d),
                  ("v", wv_d), ("vr", wvr_d)):
        w_sb[nm] = const.tile([128, KB, JW], FP8, name=f"w{nm}")
    wo_sb = const.tile([128, 4, E], BF16)

    xT = [xTp.tile([128, KB, 512], FP8, name=f"xT{si}")
          for si in range(SC)]
    xTr = [xTp.tile([128, KB, 512], FP8, name=f"xTr{si}")
           for si in range(SC)]
    # q/k per head-pair: [part = (h%2)*64 + d, slot, s]; q slots = (q8, qr8),
    # k slots = (k8, k8 duplicate) so scores DR contracts (q8+qr8) vs k8.
    qT = [qkp.tile([128, 2, S], FP8, name=f"qT{p}") for p in range(4)]
    kT = [qkp.tile([128, 2, S], FP8, name=f"kT{p}") for p in range(4)]
    # per-head v row padded to 80 so the DoubleRow weights AP k-tile
    # step (HPC*VP) is a multiple of 16 (s3_lw dual-fp8 ISA rule)
    VP = 80
    vext = vxp.tile([128, SB, HPC, VP], FP8)
    vextR = vxp.tile([128, SB, HPC, VP], FP8, name="vextR")
    concatT = ccp.tile([128, 4, S], BF16)
    attn_sF = vxp.tile([128, SB, 2, DH], BF16, name="attn_sF")

    # spread input loads over four DMA queues so the first projection can
    # start ~3.5us in instead of ~17us (single-queue serialization)
    for si in range(SC):
        for t_, d_ in ((xT, xt_d), (xTr, xtr_d)):
            nc.sync.dma_start(
                t_[si][:],
                d_.rearrange("(eb p) s -> p eb s", p=128)[
                    :, :, si * 512:(si + 1) * 512])
    nc.sync.dma_start(bq_sb[:], bq_d[:])
    nc.sync.dma_start(bk_sb[:], bk_d[:])
    nc.sync.dma_start(ident[:], id_d[:])
    qmap = {"k": nc.scalar, "kr": nc.scalar, "q": nc.scalar,
            "qr": nc.scalar, "v": nc.gpsimd, "vr": nc.gpsimd}
    for nm, d in (("k", wk_d), ("kr", wkr_d), ("q", wq_d), ("qr", wqr_d),
                  ("v", wv_d), ("vr", wvr_d)):
        qmap[nm].dma_start(w_sb[nm][:],
                           d.rearrange("(eb p) j -> p eb j", p=128))
    nc.scalar.dma_start(wo_sb[:],
                        wo_d.rearrange("(fb p) e -> p fb e", p=128))
    nc.vector.memset(vext[:, :, :, DH:DH + 1], 1.0)

    # ---- engine-dispatched element ops ----
    # NOTE: GPSIMD/Pool cannot access PSUM (BIR verifier), so every op that
    # reads a matmul result is ACT or DVE; Pool gets the SBUF-side work.
    unit_ctr = [0]

    def d_exp(dst, scp):
        ee = "A" if unit_ctr[0] < WARM_UNITS else EXP_ENGINES
        e = sched.pick(ee, 512)
        if e == "A":
            nc.scalar.activation(dst, scp[:], Exp, scale=EXP_SCALE)
        else:
            nc.vector.tensor_scalar(dst.bitcast(I8), scp[:], SCH_A, SCH_B,
                                    MULT, ADD)

    def d_exp2(dst, scp):
        e = sched.pick(EXP_ENGINES, 1024)
        if e == "A":
            nc.scalar.activation(dst, scp[:], Exp, scale=EXP_SCALE)
        else:
            nc.vector.tensor_scalar(dst.bitcast(I8), scp[:], SCH_A, SCH_B,
                                    MULT, ADD)

    def d_ts(dst, src, scale):
        e = sched.pick("AD", 512)
        if e == "A":
            nc.scalar.activation(dst, src, Copy, scale=scale)
        else:
            nc.vector.tensor_scalar(dst, src, scale, None, MULT)

    def d_res(dst, src, scale, base):
        """dst = fp8(src*scale - base); needs a tensor operand -> DVE only"""
        sched.charge("D", 512)
        nc.vector.scalar_tensor_tensor(dst, src, scale, base, MULT, SUB)

    def d_outcopy(dst, po):
        e = sched.pick("AD", 512)
        if e == "A":
            nc.scalar.activation(dst, po[:], Copy)
        else:
            nc.vector.tensor_copy(dst, po[:])

    def d_copy(dst, src, cols):
        e = sched.pick("AD", cols)
        if e == "A":
            nc.scalar.activation(dst, src, Copy)
        else:
            nc.vector.tensor_copy(dst, src)

    # ---- PE work units ----
    def proj_qk(wn, wrn, dT, b_sb, pair, sc, dup_k):
        def emit():
            pq = ps.tile([128, 512], F32, tag="ps", name=f"p{wn}{pair}_{sc}")
            wslice = slice(pair * 128, (pair + 1) * 128)
            plan = [(xT[sc], w_sb[wn]), (xTr[sc], w_sb[wn]),
                    (xT[sc], w_sb[wrn])]
            n = 0
            for rhs_t, w in plan:
                for kb in range(4):
                    n += 1
                    nc.tensor.matmul(pq[:],
                                     w[:, 2 * kb:2 * kb + 2, wslice],
                                     rhs_t[:, 2 * kb:2 * kb + 2, :],
                                     start=(n == 1), stop=False,
                                     perf_mode=DR)
            # rank-1 bias row (exact bias, fp8 noise ~0.1%): += 32b ⊗ ones
            nc.tensor.matmul(pq[:], b_sb[:, wslice], ones_r[:],
                             start=False, stop=True)
            dst0 = dT[pair][:, 0, sc * 512:(sc + 1) * 512]
            d_ts(dst0, pq[:], QK_OUT)
            if dup_k:
                # k8 duplicate for the DoubleRow slot pair: Pool is idle and
                # its queue is short (SBUF->SBUF legal for gpsimd)
                nc.gpsimd.tensor_scalar(
                    dT[pair][:, 1, sc * 512:(sc + 1) * 512], dst0, 0.0,
                    None, ADD)
                sched.charge("P", 512)
            else:
                d_res(dT[pair][:, 1, sc * 512:(sc + 1) * 512], pq[:],
                      QK_OUT, dst0)
        return emit

    def v_chunk(tb):
        def emit():
            pv = ps.tile([128, 512], F32, tag="ps", name=f"pv{tb}")
            si, to = tb // 4, (tb % 4) * 128
            ts_ = slice(to, to + 128)
            plan = [(xT[si], w_sb["v"]), (xTr[si], w_sb["v"]),
                    (xT[si], w_sb["vr"])]
            n = 0
            for lhs_t, w in plan:
                for kb in range(4):
                    n += 1
                    nc.tensor.matmul(pv[:],
                                     lhs_t[:, 2 * kb:2 * kb + 2, ts_],
                                     w[:, 2 * kb:2 * kb + 2, :],
                                     start=(n == 1), stop=(n == 12),
                                     perf_mode=DR)
            dst0 = vext[:, tb, :, 0:DH]
            pvr = pv[:].rearrange("p (h d) -> p h d", h=HPC)
            d_ts(dst0, pvr, V_OUT)
            d_res(vextR[:, tb, :, 0:DH], pvr, V_OUT, dst0)
        return emit

    def outproj_chunk(sb, ec):
        def emit():
            po = ps.tile([128, 512], F32, tag="ps", name=f"po{sb}_{ec}")
            for fb in range(4):
                nc.tensor.matmul(po[:],
                                 concatT[:, fb, sb * 128:(sb + 1) * 128],
                                 wo_sb[:, fb, ec * 512:(ec + 1) * 512],
                                 start=(fb == 0), stop=(fb == 3))
            ot = outp.tile([128, 512], F32, tag="ot", name=f"ot{sb}_{ec}")
            d_outcopy(ot[:], po)
            nc.sync.dma_start(
                out_d[sb * 128:(sb + 1) * 128, ec * 512:(ec + 1) * 512],
                ot[:])
        return emit

    def transp_chunk(acc, pair, sc, i):
        def emit():
            sb = 4 * sc + i
            nc.sync.dma_start_transpose(
                concatT[:, pair, sb * 128:(sb + 1) * 128],
                attn_sF[:, sb, :, :])
        return emit

    def normalize_smaj(acc, h, sc):
        accS = nrm.tile([128, 260], F32, tag="accSq")
        d_copy(accS[:], acc[:, 0:260], 260)
        for i in range(4):
            sb = 4 * sc + i
            nc.gpsimd.normalize_recip(
                attn_sF[:, sb, h % 2, :],
                accS[:, 65 * i:65 * i + 64],
                accS[:, 65 * i + 64:65 * i + 65])
            sched.charge("P", 64)
        if h % 2 == 1:
            for i in range(4):
                reg(("tp", h // 2, sc, i), transp_chunk(acc, h // 2, sc, i))

    def normalize(acc, h, sc):
        # hop 1 frees the PSUM acc bank fast; the rest is slack-tolerant
        # (concatT is only read by outproj at s-chunk end).
        accS = nrm.tile([DH + 1, 512], F32, tag="accS")
        d_outcopy(accS[:], acc)
        r_t = nrm.tile([1, 512], F32, tag="r")
        nc.vector.reciprocal(r_t[:], accS[DH:DH + 1, :])
        rb_t = nrm.tile([64, 512], F32, tag="rb")
        nc.gpsimd.partition_broadcast(rb_t[:], r_t[:])
        sched.charge("D", 512)
        sched.charge("P", 512)
        bp = (h % 2) * 64
        nc.gpsimd.tensor_tensor(
            concatT[bp:bp + 64, h // 2, sc * 512:(sc + 1) * 512],
            accS[0:DH, :], rb_t[:], MULT)
        sched.charge("P", 512)
        sched.charge("P", 512)  # Multiply runs at 0.42 eff on Q7

    # ---- chunk registry: emit-on-demand + background fill order ----
    chunks = {}
    fill_order = deque()

    def reg(key, fn):
        chunks[key] = fn
        fill_order.append(key)

    def ensure(key):
        fn = chunks.pop(key, None)
        if fn is not None:
            fn()

    def fill(n):
        for _ in range(n):
            while fill_order and fill_order[0] not in chunks:
                fill_order.popleft()
            if not fill_order:
                return
            ensure(fill_order.popleft())

    for scp_ in range(SC):
        for pair in range(4):
            reg(("k", pair, scp_),
                proj_qk("k", "kr", kT, bk_sb, pair, scp_, True))
            reg(("q", pair, scp_),
                proj_qk("q", "qr", qT, bq_sb, pair, scp_, False))
    for tb in range(SB):
        reg(("v", tb), v_chunk(tb))

    def ensure_qk(pair, sc_q, tbmax):
        ensure(("q", pair, sc_q))
        for scp_ in range(min(tbmax // 4 + 2, SC)):
            ensure(("k", pair, scp_))

    # ---- main attention stream: s-chunk outer, head inner ----
    pend = deque()

    def attn_step_smaj(acc, ex_t, h, g, sc):
        def emit():
            ensure(("v", 2 * g))
            ensure(("v", 2 * g + 1))
            vA = vext[:, 2 * g:2 * g + 2, h, 0:DH + 1]
            vB = vextR[:, 2 * g:2 * g + 2, h, 0:DH]
            kw = dict(perf_mode=DR, skip_group_check=True)
            # PSUM zeroing is 2KB-bank-granular: exactly ONE start (zeroes
            # the whole bank = all four sub-accs) and ONE stop per bank.
            for i in range(4):
                exl = ex_t[:, :, i * 128:(i + 1) * 128]
                oA = acc[:, 65 * i:65 * i + 65]
                oB = acc[:, 65 * i:65 * i + 64]
                if g == 7:
                    nc.tensor.matmul(oB, exl, vB, start=False, stop=False,
                                     **kw)
                    nc.tensor.matmul(oA, exl, vA, start=False,
                                     stop=(i == 3), **kw)
                else:
                    nc.tensor.matmul(oA, exl, vA,
                                     start=(g == 0 and i == 0),
                                     stop=False, **kw)
                    nc.tensor.matmul(oB, exl, vB, start=False, stop=False,
                                     **kw)
            if g == 7:
                normalize_smaj(acc, h, sc)
                if h == HPC - 1:
                    for sb in range(4 * sc, 4 * sc + 4):
                        reg(("o", sb, 0), outproj_chunk(sb, 0))
                        reg(("o", sb, 1), outproj_chunk(sb, 1))
        return emit

    def attn_step(acc, ex_t, h, g, sc):
        def emit():
            ensure(("v", 2 * g))
            ensure(("v", 2 * g + 1))
            # main (v8+ones) instr opens the group at g0 (zeroing the full
            # bank incl. the sums row) and closes it at g7; the residual
            # (vr8) instr accumulates rows 0..63 strictly inside the group.
            a_args = dict(start=(g == 0), stop=(g == 7), perf_mode=DR,
                          skip_group_check=True)
            b_args = dict(start=False, stop=False, perf_mode=DR,
                          skip_group_check=True)
            if g == 7:
                nc.tensor.matmul(acc[0:DH, :],
                                 vextR[:, 2 * g:2 * g + 2, h, 0:DH],
                                 ex_t[:], **b_args)
                nc.tensor.matmul(acc[:], vext[:, 2 * g:2 * g + 2, h, 0:DH + 1],
                                 ex_t[:], **a_args)
            else:
                nc.tensor.matmul(acc[:], vext[:, 2 * g:2 * g + 2, h, 0:DH + 1],
                                 ex_t[:], **a_args)
                nc.tensor.matmul(acc[0:DH, :],
                                 vextR[:, 2 * g:2 * g + 2, h, 0:DH],
                                 ex_t[:], **b_args)
            if g == 7:
                normalize(acc, h, sc)
                if h == HPC - 1:
                    for sb in range(4 * sc, 4 * sc + 4):
                        reg(("o", sb, 0), outproj_chunk(sb, 0))
                        reg(("o", sb, 1), outproj_chunk(sb, 1))
        return emit

    # head-outer spreads the k/v projection front-load across the run
    # (pair p's k chunks are pulled by head 2p); out-projections still
    # unlock per s-chunk because head 7 runs last.
    for h in range(HPC):
        for sc in range(SC):
            pair, bp = h // 2, (h % 2) * 64
            # prefetch upcoming projections so their fp8 copies clear the
            # ACT/DVE queues before the scores that read them
            if sc < SC - 1:
                ensure(("q", pair, sc + 1))
            elif h % 2 == 0:
                ensure(("q", pair, 0))
            elif h < HPC - 1:
                ensure(("q", pair + 1, 0))
                for sp_ in range(SC):
                    ensure(("k", pair + 1, sp_))
            if ATTN_SMAJ:
                acc = psa.tile([128, 512], F32, tag="acc",
                               name=f"acc{h}_{sc}")
            else:
                acc = psa.tile([DH + 1, 512], F32, tag="acc",
                               name=f"acc{h}_{sc}")
            for g in range(8):
                ex_t = exp_p.tile([128, 2, 512], FP8, tag="ex",
                                  name=f"ex{h}_{sc}_{g}")
                if SCP_PAIR:
                    scp = ps.tile([128, 2, 512], F32, tag="ps2",
                                  bufs=SCP2_BUFS, name=f"scp{h}_{sc}_{g}")
                for t2 in range(2):
                    tb = 2 * g + t2
                    ensure_qk(pair, sc, tb)
                    if not SCP_PAIR:
                        scp = ps.tile([128, 512], F32, tag="ps",
                                      name=f"scp{h}_{sc}_{tb}")
                    dst = scp[:, t2, :] if SCP_PAIR else scp[:]
                    nc.tensor.matmul(
                        dst,
                        kT[pair][bp:bp + 64, :, tb * 128:(tb + 1) * 128],
                        qT[pair][bp:bp + 64, :, sc * 512:(sc + 1) * 512],
                        start=True, stop=True, perf_mode=DR)
                    if not SCP_PAIR:
                        d_exp(ex_t[:, t2, :], scp)
                if SCP_PAIR:
                    d_exp2(ex_t[:], scp)
                unit_ctr[0] += 1
                pend.append((attn_step_smaj if ATTN_SMAJ else attn_step)(
                    acc, ex_t, h, g, sc))
                fill(1)
                while len(pend) > LAG:
                    pend.popleft()()
    while pend:
        pend.popleft()()
    fill(len(fill_order))


_CACHE = {}


def _build():
    nc = bacc.Bacc("TRN2", target_bir_lowering=False, debug=False,
                   num_devices=N_CORES)
    names = [("xt", [E, S], FP8), ("xtr", [E, S], FP8),
             ("wq", [E, JW], FP8), ("wqr", [E, JW], FP8),
             ("wk", [E, JW], FP8), ("wkr", [E, JW], FP8),
             ("wv", [E, JW], FP8), ("wvr", [E, JW], FP8),
             ("wo", [JW, E], BF16),
             ("bq", [1, JW], FP8), ("bk", [1, JW], FP8),
             ("ident", [128, 128], BF16)]
    aps = tuple(nc.dram_tensor(n, s, d, kind="ExternalInput").ap()
                for n, s, d in names)
    out_d = nc.dram_tensor("out", [S, E], F32, kind="ExternalOutput").ap()
    with tile.TileContext(nc) as tc:
        with ExitStack() as ctx:
            _emit(tc, aps + (out_d,), ctx)
    nc.compile()
    return nc


FP8NP = ml_dtypes.float8_e4m3
BF16NP = ml_dtypes.bfloat16


def _two_term(a):
    """fp8 value + fp8 residual of array a."""
    a8 = a.astype(FP8NP)
    r8 = (a - a8.astype(np.float32)).astype(FP8NP)
    return a8, r8


def kernel(x, Wq, bq, Wk, bk, Wv, bv, Wo, bo):
    x = np.asarray(x, dtype=np.float32)
    Wq = np.asarray(Wq, dtype=np.float32)
    bq = np.asarray(bq, dtype=np.float32)
    Wk = np.asarray(Wk, dtype=np.float32)
    bk = np.asarray(bk, dtype=np.float32)
    Wv = np.asarray(Wv, dtype=np.float32)
    bv = np.asarray(bv, dtype=np.float32)
    Wo = np.asarray(Wo, dtype=np.float32)
    bo = np.asarray(bo, dtype=np.float32)

    if "nc" not in _CACHE:
        _CACHE["nc"] = _build()
    nc = _CACHE["nc"]

    WoT = np.ascontiguousarray(Wo.T)  # [f, e]
    # v-bias is linear through attention: its output contribution is the
    # constant row bv_flat @ Wo^T, added host-side.
    badj = bo + bv.reshape(H * DH) @ WoT

    in_maps = []
    for c in range(N_CORES):
        b, hh = c // 2, c % 2
        hs = slice(hh * HPC, (hh + 1) * HPC)
        xt8, xtr8 = _two_term(np.ascontiguousarray(x[b].T))
        m = {"xt": xt8, "xtr": xtr8,
             "wo": np.ascontiguousarray(
                 WoT[hh * JW:(hh + 1) * JW]).astype(BF16NP),
             "bq": (W_SCALE * bq[hs].reshape(1, JW)).astype(FP8NP),
             "bk": (W_SCALE * bk[hs].reshape(1, JW)).astype(FP8NP),
             "ident": np.eye(128, dtype=np.float32).astype(BF16NP)}
        for nm, W in (("q", Wq), ("k", Wk), ("v", Wv)):
            flat = np.ascontiguousarray(
                W[hs].transpose(1, 0, 2).reshape(E, JW)) * W_SCALE
            m["w" + nm], m["w" + nm + "r"] = _two_term(flat)
        in_maps.append(m)

    res = bass_utils.run_bass_kernel_spmd(nc, in_maps,
                                          core_ids=list(range(N_CORES)))
    out = np.empty((B, S, E), dtype=np.float32)
    for b in range(B):
        out[b] = res.results[2 * b]["out"] + res.results[2 * b + 1]["out"]
        out[b] += badj[None, :]
    return out
